# revision 13
# baseline (speedup 1.0000x reference)
"""GraphSAGE 2-block GNN (nn_BaselineModel_80607946211554) on 8 TRN2 NeuronCores.

Strategy: destination-node sharding, bf16 compute. Q7 SWDGE descriptor
generation is the hard bottleneck (~2.4ns/index + ~2.5us/instruction,
serialized on the GpSimd engine), so gathers are issued as few large
instructions: windows are batched into groups of 4 and each group's
lo/hi table section is fetched with a single dma_gather. Node tables
live in DRAM in a chunk-major layout (3 row-chunks of 2176 rows/core)
so the inter-conv AllGather can be fired in 3 pipelined pieces as soon
as the producing windows retire. Aggregation: ones-block stage-1 matmul
(sums per-dst slot quadruples, PE column-tiled) + streamed indicator
stage-2 matmul; PSUM->SBUF casts and bias+relu run on the Scalar (ACT)
engine to keep DVE free. SAGE linears run feature-major (weights
stationary); JK linear + graph pooling are fused into conv tails. The
tiny MLP head + softmax runs redundantly on every core in fp32.

Self-contained: hardcodes all shapes for the fixed problem instance.
"""
import os
import sys
import types
import numpy as np

N = 50000
E = 1600000
G = 256
F = 128
HID = 128
C = 10
NCORES = 8
NPC = N // NCORES            # 6250 nodes per core
P = 128
NW = (NPC + P - 1) // P      # 49 dst windows per core
GW = 4                       # windows per gather group
EPS = 1e-5

# chunk-major table layout: 3 chunks per core, CHKB rows each
CHKB = 2176                  # buffer rows per (core, chunk)
CHKR = (2048, 2048, NPC - 4096)   # real rows per chunk (2048, 2048, 2154)
CSLAB = 3 * CHKB             # 6528 contrib rows per core
NT = NCORES * CHKB * 3       # 52224 table rows
CHUNK_T = NCORES * CHKB      # 17408 table rows per chunk
ZROW = (2048, 2048, CHKR[2])   # guaranteed-zero chunk-local row per chunk
AGW = (16, 32, NW)           # window counts gating the 3 AllGather chunks
NSEC = 3                     # gather sections per window (one per table chunk)

_prog_cache = {}


def _bf16(a):
    import ml_dtypes
    return np.asarray(a, np.float32).astype(ml_dtypes.bfloat16)


def _trow(s):
    """Global node id -> chunk-major table row (vectorized)."""
    c, r = s // NPC, s % NPC
    k = np.minimum(r >> 11, 2)
    rr = r - (k << 11)
    return k * CHUNK_T + c * CHKB + rr


def _wrap_idx(sec):
    """int16 row values (len mult of 16) -> [128, n/16] wrapped layout."""
    n = len(sec)
    arr = sec.reshape(n // 16, 16).T.astype(np.int16)
    return np.tile(arr, (8, 1))


# ----------------------------------------------------------------- host prep
def _build_schedule(src, dst, invd_full):
    """Static schedule shared by all 4 convs + per-core gather/indicator data.

    Group layout (GW windows): [L(w0)..L(w3) | H(w0)..H(w3)], each window
    section per-dst padded to x4 then to x128 (pads gather a zero row).
    Stage-1 sums slot quadruples into blocks; stage-2 is one indicator
    matmul per (128-block tile x window segment)."""
    groups = [list(range(w0, min(w0 + GW, NW))) for w0 in range(0, NW, GW)]

    # --- per-core edge prep
    core_data = []
    nS = np.zeros((NCORES, NW, NSEC), np.int64)
    for c in range(NCORES):
        m = (dst >= c * NPC) & (dst < (c + 1) * NPC)
        s = src[m].astype(np.int64)
        d = (dst[m] - c * NPC).astype(np.int64)
        t = _trow(s)
        sec = t // CHUNK_T
        val = t % CHUNK_T
        w = d >> 7
        order = np.lexsort((d, sec, w))
        s, d, sec, w, val = s[order], d[order], sec[order], w[order], val[order]
        cnt = np.bincount(d * NSEC + sec, minlength=NPC * NSEC).reshape(NPC, NSEC)
        pl = ((cnt + 3) >> 2) << 2          # per-(dst,section) slots, x4
        plp = np.zeros((NW * P, NSEC), np.int64)
        plp[:NPC] = pl
        plw = plp.reshape(NW, P, NSEC).sum(1)
        nS[c] = ((plw + 127) >> 7) << 7
        core_data.append((s, d, sec, w, val, pl))

    nSm = np.maximum(nS.max(0), P)   # [NW, NSEC] shared across cores

    # --- group geometry (shared); sections padded to x512 slots
    ginfo = []
    icol = 0
    mm_total = 0
    for grp in groups:
        secs = []
        for k in range(NSEC):
            raw = int(nSm[grp, k].sum())
            n = ((raw + 511) // 512) * 512
            ncols = n // P
            ntiles = ncols // 4
            colw = []
            for w in grp:
                colw += [w] * (int(nSm[w, k]) // P)
            colw += [-1] * (ncols - len(colw))
            blkw = np.repeat(np.array(colw, np.int64), 32)
            mms = []
            for t in range(ntiles):
                bw = blkw[t * 128:(t + 1) * 128]
                runs = []
                prev = None
                for b in range(128):
                    if bw[b] != prev:
                        runs.append([int(bw[b]), b, b + 1])
                        prev = bw[b]
                    else:
                        runs[-1][2] = b + 1
                for wv, b0, b1 in runs:
                    if wv < 0:
                        continue
                    mms.append((t, wv, b0, b1, mm_total))
                    mm_total += 1
            secs.append(dict(n=n, raw=raw, ncols=ncols, ntiles=ntiles,
                             mms=mms, icol=icol,
                             mi0=mms[0][4] if mms else mm_total, nmm=len(mms)))
            icol += n // 16
        first = {}
        last = {}
        for sec in secs:
            for (t, wv, b0, b1, mi) in sec["mms"]:
                if wv not in first:
                    first[wv] = mi
                last[wv] = mi
        ginfo.append(dict(grp=grp, secs=secs, first=first, last=last))
    idx_cols = icol
    sec_cols = max(sec["ncols"] for g in ginfo for sec in g["secs"])
    i2max = max(sec["nmm"] for g in ginfo for sec in g["secs"])

    # --- per-core idx + indicator data
    per_core = []
    for c in range(NCORES):
        s, d, sec, w, val, pl = core_data[c]
        plw3 = np.zeros((NW * P, NSEC), np.int64)
        plw3[:NPC] = pl
        plw3 = plw3.reshape(NW, P, NSEC)
        dstart = np.cumsum(plw3, axis=1) - plw3      # [NW,128,NSEC]
        key = d * NSEC + sec
        if len(key):
            grp_change = np.r_[True, key[1:] != key[:-1]]
            first_pos = np.flatnonzero(grp_change)
            gidx = np.cumsum(grp_change) - 1
            rank = np.arange(len(d)) - first_pos[gidx]
        else:
            rank = np.zeros(0, np.int64)
        pos = dstart[w, d & 127, sec] + rank         # slot within section

        idx_arr = np.zeros((P, idx_cols), np.int16)
        ind2 = np.zeros((P, mm_total, P), np.float32)   # [block, mm, dst]
        for gi, g in enumerate(ginfo):
            grp = g["grp"]
            for k in range(NSEC):
                sk = g["secs"][k]
                off = {}
                o = 0
                for wv in grp:
                    off[wv] = o
                    o += int(nSm[wv, k])
                seck = np.full(sk["n"], ZROW[k], np.int64)
                for wv in grp:
                    mk = (w == wv) & (sec == k)
                    seck[off[wv] + pos[mk]] = val[mk]
                idx_arr[:, sk["icol"]:sk["icol"] + sk["n"] // 16] = _wrap_idx(seck)

                b2d = np.full(sk["ntiles"] * 128, -1, np.int64)
                bo = 0
                for wv in grp:
                    d0, d1 = wv * P, min((wv + 1) * P, NPC)
                    dloc = np.arange(d1 - d0)
                    bK = np.repeat(dloc, pl[d0:d1, k] // 4)
                    b2d[bo:bo + len(bK)] = bK
                    bo += int(nSm[wv, k]) // 4
                for (t, wv, b0, b1, mi) in sk["mms"]:
                    bs = b2d[t * 128 + b0:t * 128 + b1]
                    rows = np.arange(b0, b1)
                    vm = bs >= 0
                    ind2[rows[vm], mi, bs[vm]] = invd_full[
                        c * NPC + wv * P + bs[vm]]
        per_core.append(dict(idx=idx_arr, ind2=_bf16(ind2)))

    sched = dict(ginfo=ginfo, idx_cols=idx_cols, mm_total=mm_total,
                 sec_cols=sec_cols, i2max=i2max,
                 key=(idx_cols, mm_total, sec_cols, i2max,
                      tuple(int(x) for x in nSm.reshape(-1))))
    return sched, per_core


def _host_inputs(inputs):
    f32 = lambda a: np.asarray(a, np.float32)
    x = f32(inputs["x"])
    ei = np.asarray(inputs["edge_index"], np.int64)
    batch = np.asarray(inputs["batch"], np.int64)
    src, dst = ei[0], ei[1]

    deg = np.bincount(dst, minlength=N).astype(np.float32)
    invd_full = (1.0 / np.maximum(deg, 1.0)).astype(np.float32)

    sched, per_core = _build_schedule(src, dst, invd_full)

    # x in chunk-major table layout
    xt = np.zeros((NT, F), np.float32)
    xt[_trow(np.arange(N))] = x

    o4 = np.zeros((P, 32), np.float32)
    for e in range(P):
        o4[e, e // 4] = 1.0
    ident = np.eye(P, dtype=np.float32)

    # BN folding
    s_bn = f32(inputs["bn_gamma"]) / np.sqrt(f32(inputs["bn_rv"]) + EPS)
    t_bn = f32(inputs["bn_beta"]) - f32(inputs["bn_rm"]) * s_bn
    bns2 = s_bn.reshape(2, P).T.copy()     # [128, 2]
    bnt2 = t_bn.reshape(2, P).T.copy()

    shared = {
        "xt": _bf16(xt), "o4": _bf16(o4), "ident": _bf16(ident),
        "ident32": ident,
        "bns2": bns2, "bnt2": bnt2,
        "l1w": f32(inputs["lin1_W"]), "l1b": f32(inputs["lin1_b"]),
        "l2w": f32(inputs["lin2_W"]), "l2b": f32(inputs["lin2_b"]),
    }
    for b in (0, 1):
        for nm in ("Wl1", "Wr1", "Wl2", "Wr2", "Wlin"):
            shared[f"b{b}_{nm}"] = _bf16(inputs[f"b{b}_{nm}"])
        for nm in ("b1", "b2", "blin"):
            shared[f"b{b}_{nm}"] = f32(inputs[f"b{b}_{nm}"])

    in_maps = []
    for c in range(NCORES):
        xoT = np.zeros((F, NW * P), np.float32)
        xoT[:, :NPC] = x[c * NPC:(c + 1) * NPC].T
        pool_ind = np.zeros((NW, P, G), np.float32)
        bt = batch[c * NPC:(c + 1) * NPC]
        btp = np.full(NW * P, -1, np.int64)
        btp[:NPC] = bt
        btp2 = btp.reshape(NW, P)
        for wi in range(NW):
            vmb = btp2[wi] >= 0
            pool_ind[wi, np.arange(P)[vmb], btp2[wi][vmb]] = 1.0
        im = dict(shared)
        im.update({
            "xoT": _bf16(xoT), "poolind": _bf16(pool_ind),
            "idx": per_core[c]["idx"], "ind2": per_core[c]["ind2"],
        })
        in_maps.append(im)
    return sched, in_maps


# ------------------------------------------------------------- bass program
def _build_program(sched, n_convs=4, debug_tables=False):
    import concourse.bass as bass
    import concourse.mybir as mybir
    import concourse.tile as tile
    from concourse import bacc
    from concourse import library_config
    from contextlib import ExitStack

    dt = mybir.dt
    DT = dt.bfloat16
    F32 = dt.float32
    Alu = mybir.AluOpType
    Act = mybir.ActivationFunctionType

    ginfo = sched["ginfo"]
    SECMAX = sched["sec_cols"]
    I2MAX = sched["i2max"]

    nc = bacc.Bacc("TRN2", debug=False, num_swdge_queues=4)

    # ---- parameters
    xt = nc.declare_dram_parameter("xt", [NT, F], DT, isOutput=False)
    xoT = nc.declare_dram_parameter("xoT", [F, NW * P], DT, isOutput=False)
    idxp = nc.declare_dram_parameter("idx", [P, sched["idx_cols"]], dt.int16, isOutput=False)
    ind2p = nc.declare_dram_parameter("ind2", [P, sched["mm_total"], P], DT, isOutput=False)
    poolp = nc.declare_dram_parameter("poolind", [NW, P, G], DT, isOutput=False)
    o4p = nc.declare_dram_parameter("o4", [P, 32], DT, isOutput=False)
    identp = nc.declare_dram_parameter("ident", [P, P], DT, isOutput=False)
    ident32p = nc.declare_dram_parameter("ident32", [P, P], F32, isOutput=False)
    wp = {}
    for b in (0, 1):
        for nm, shp, dty in (("Wl1", [F, HID], DT), ("Wr1", [F, HID], DT),
                             ("b1", [HID], F32),
                             ("Wl2", [HID, HID], DT), ("Wr2", [HID, HID], DT),
                             ("b2", [HID], F32),
                             ("Wlin", [2 * HID, HID], DT), ("blin", [HID], F32)):
            wp[f"b{b}_{nm}"] = nc.declare_dram_parameter(f"b{b}_{nm}", shp, dty, isOutput=False)
    bns2p = nc.declare_dram_parameter("bns2", [P, 2], F32, isOutput=False)
    bnt2p = nc.declare_dram_parameter("bnt2", [P, 2], F32, isOutput=False)
    l1wp = nc.declare_dram_parameter("l1w", [2 * HID, HID], F32, isOutput=False)
    l1bp = nc.declare_dram_parameter("l1b", [HID], F32, isOutput=False)
    l2wp = nc.declare_dram_parameter("l2w", [HID, C], F32, isOutput=False)
    l2bp = nc.declare_dram_parameter("l2b", [C], F32, isOutput=False)

    out = nc.declare_dram_parameter("out", [G, C], F32, isOutput=True)
    if debug_tables:
        dbgA = nc.declare_dram_parameter("dbgA", [NT, F], DT, isOutput=True)
        dbgB = nc.declare_dram_parameter("dbgB", [NT, F], DT, isOutput=True)

    with tile.TileContext(nc) as tc, ExitStack() as ctx:
        sb = ctx.enter_context(tc.tile_pool(name="sb", bufs=1))
        sb_feat = ctx.enter_context(tc.tile_pool(name="sb_feat", bufs=1))
        sb_g = ctx.enter_context(tc.tile_pool(name="sb_g", bufs=4))
        sb_i2 = ctx.enter_context(tc.tile_pool(name="sb_i2", bufs=4))
        sb_bs = ctx.enter_context(tc.tile_pool(name="sb_bs", bufs=6))
        sb_ms = ctx.enter_context(tc.tile_pool(name="sb_ms", bufs=3))
        ps_bs = ctx.enter_context(tc.tile_pool(name="ps_bs", bufs=3, space="PSUM"))
        ps_agg = ctx.enter_context(tc.tile_pool(name="ps_agg", bufs=2, space="PSUM"))
        ps_mm = ctx.enter_context(tc.tile_pool(name="ps_mm", bufs=2, space="PSUM"))
        ps_pool = ctx.enter_context(tc.tile_pool(name="ps_pool", bufs=1, space="PSUM"))
        dram = ctx.enter_context(tc.tile_pool(name="dram", bufs=1, space="DRAM"))

        nc.gpsimd.load_library(library_config.mlp)

        # ---- constants into SBUF
        o4_t = sb.tile([P, 32], DT, name="o4_t", tag="o4_t")
        nc.sync.dma_start(o4_t[:], o4p[:])
        id_t = sb.tile([P, P], DT, name="id_t", tag="id_t")
        nc.sync.dma_start(id_t[:], identp[:])
        id32_t = sb.tile([P, P], F32, name="id32_t", tag="id32_t")
        nc.sync.dma_start(id32_t[:], ident32p[:])
        idx_t = sb.tile([P, sched["idx_cols"]], dt.int16, name="idx_t", tag="idx_t")
        nc.sync.dma_start(idx_t[:], idxp[:])


        wt = {}
        for b in (0, 1):
            for nm in ("Wl1", "Wr1", "Wl2", "Wr2"):
                w_t = sb.tile([P, P], DT, name=f"w{b}{nm}", tag=f"w{b}{nm}")
                nc.sync.dma_start(w_t[:], wp[f"b{b}_{nm}"][:])
                wt[f"b{b}_{nm}"] = w_t
            wlin_t = sb.tile([P, 2, P], DT, name=f"w{b}lin", tag=f"w{b}lin")
            nc.sync.dma_start(wlin_t[:, 0, :], wp[f"b{b}_Wlin"][0:P, :])
            nc.sync.dma_start(wlin_t[:, 1, :], wp[f"b{b}_Wlin"][P:2 * P, :])
            wt[f"b{b}_Wlin"] = wlin_t
            for nm in ("b1", "b2", "blin"):
                b_t = sb.tile([P, 1], F32, name=f"b{b}{nm}", tag=f"b{b}{nm}")
                nc.sync.dma_start(b_t[:], wp[f"b{b}_{nm}"][:, None])
                wt[f"b{b}_{nm}"] = b_t
        bns_t = sb.tile([P, 2], F32, name="bns_t", tag="bns_t")
        nc.sync.dma_start(bns_t[:], bns2p[:])
        bnt_t = sb.tile([P, 2], F32, name="bnt_t", tag="bnt_t")
        nc.sync.dma_start(bnt_t[:], bnt2p[:])
        l1w_t = sb.tile([P, 2, P], F32, name="l1w_t", tag="l1w_t")
        nc.sync.dma_start(l1w_t[:, 0, :], l1wp[0:P, :])
        nc.sync.dma_start(l1w_t[:, 1, :], l1wp[P:2 * P, :])
        l1b_t = sb.tile([P, 1], F32, name="l1b_t", tag="l1b_t")
        nc.sync.dma_start(l1b_t[:], l1bp[:, None])
        l2w_t = sb.tile([P, C], F32, name="l2w_t", tag="l2w_t")
        nc.sync.dma_start(l2w_t[:], l2wp[:])
        l2b_t = sb.tile([P, 1], F32, name="l2b_t", tag="l2b_t")
        nc.sync.dma_start(l2b_t[0:C, :], l2bp[:, None])

        # feature-major activation buffers [128, NW*128] bf16
        featA = sb_feat.tile([P, NW * P], DT, name="featA", tag="featA")
        featB = sb_feat.tile([P, NW * P], DT, name="featB", tag="featB")
        featC = sb_feat.tile([P, NW * P], DT, name="featC", tag="featC")
        nc.sync.dma_start(featA[:], xoT[:])

        zero_t = sb.tile([P, P], DT, name="zero_t", tag="zero_t")
        nc.vector.memset(zero_t[:], 0.0)

        # DRAM scratch: contribs (chunk-padded) + tables
        cA = dram.tile([CSLAB, F], DT, name="cA", tag="cA")
        cB = dram.tile([CSLAB, F], DT, name="cB", tag="cB")
        tabA = [dram.tile([CHUNK_T, F], DT, addr_space="Shared",
                          name=f"tabA{k}", tag=f"tabA{k}") for k in range(3)]
        tabB = [dram.tile([CHUNK_T, F], DT, addr_space="Shared",
                          name=f"tabB{k}", tag=f"tabB{k}") for k in range(3)]
        tabC = [dram.tile([CHUNK_T, F], DT, addr_space="Shared",
                          name=f"tabC{k}", tag=f"tabC{k}") for k in range(3)]
        pc_in = dram.tile([P, 2 * G], F32, name="pc_in", tag="pc_in")
        pc_out = dram.tile([P, 2 * G], F32, addr_space="Shared", name="pc_out", tag="pc_out")
        # zero the per-chunk pad rows of the contribs
        for cbuf in (cA, cB):
            for k in range(3):
                z0, z1 = k * CHKB + CHKR[k], (k + 1) * CHKB
                r = z0
                while r < z1:
                    n = min(P, z1 - r)
                    nc.sync.dma_start(cbuf[r:r + n, :], zero_t[0:n, :])
                    r += n

        # tiny warmup collective
        wu_in = dram.tile([P, 8], DT, name="wu_in", tag="wu_in")
        wu_out = dram.tile([P, 64], DT, addr_space="Shared", name="wu_out", tag="wu_out")
        nc.sync.dma_start(wu_in[0:P, :], zero_t[0:P, 0:8])
        nc.gpsimd.collective_compute(
            "AllGather", Alu.bypass, ins=[wu_in[:]], outs=[wu_out[:]],
            replica_groups=[list(range(NCORES))])

        def win_tail(w, agg, in_feat, out_feat, Wl, Wr, bcol, contrib, jkargs,
                     pool_ps):
            """Everything after window w's aggregation closes."""
            mT_sb = sb_ms.tile([P, P], DT, name="mT_sb", tag="mT_sb")
            nc.scalar.copy(mT_sb[:], agg[:])
            h_ps = ps_mm.tile([P, P], F32, name="h_ps", tag="mm")
            nc.tensor.matmul(h_ps[:], Wl[:], mT_sb[:], start=True, stop=False)
            nc.tensor.matmul(h_ps[:], Wr[:], in_feat[:, w * P:(w + 1) * P], start=False, stop=True)
            nc.scalar.activation(out_feat[:, w * P:(w + 1) * P], h_ps[:], Act.Relu, bias=bcol[:], scale=1.0)
            if contrib is not None:
                rows = min(P, NPC - w * P)
                k = 0 if w < 16 else (1 if w < 32 else 2)
                crow = k * CHKB + (w * P - k * 2048)
                hnm_ps = ps_mm.tile([P, P], DT, name="hnm_ps", tag="mm")
                nc.tensor.transpose(hnm_ps[:], out_feat[:, w * P:(w + 1) * P], id_t[:])
                hnm_sb = sb_ms.tile([P, P], DT, name="hnm_sb", tag="hnm_sb")
                nc.vector.tensor_copy(hnm_sb[:], hnm_ps[:])
                nc.scalar.dma_start(contrib[crow:crow + rows, :], hnm_sb[0:rows, :])
            if jkargs is not None:
                Wlin, jbcol, hout, jcontrib = jkargs
                j_ps = ps_mm.tile([P, P], F32, name="jk_ps", tag="mm")
                nc.tensor.matmul(j_ps[:], Wlin[:, 0, :], in_feat[:, w * P:(w + 1) * P], start=True, stop=False)
                nc.tensor.matmul(j_ps[:], Wlin[:, 1, :], out_feat[:, w * P:(w + 1) * P], start=False, stop=True)
                nc.scalar.activation(hout[:, w * P:(w + 1) * P], j_ps[:], Act.Relu, bias=jbcol[:], scale=1.0)
                jnm_ps = ps_mm.tile([P, P], DT, name="jknm_ps", tag="mm")
                nc.tensor.transpose(jnm_ps[:], hout[:, w * P:(w + 1) * P], id_t[:])
                jnm_sb = sb_ms.tile([P, P], DT, name="jknm_sb", tag="jknm_sb")
                nc.vector.tensor_copy(jnm_sb[:], jnm_ps[:])
                if jcontrib is not None:
                    rows = min(P, NPC - w * P)
                    k = 0 if w < 16 else (1 if w < 32 else 2)
                    crow = k * CHKB + (w * P - k * 2048)
                    nc.scalar.dma_start(jcontrib[crow:crow + rows, :], jnm_sb[0:rows, :])
                pind = sb_i2.tile([P, G], DT, name="pind", tag="pind")
                nc.sync.dma_start(pind[:], poolp[w])
                nc.tensor.matmul(pool_ps[:], jnm_sb[:], pind[:],
                                 start=(w == 0), stop=(w == NW - 1))

        def conv(tab, in_feat, out_feat, Wl, Wr, bcol, contrib, ag_out=None,
                 jkargs=None):
            """One SAGE conv. contrib: chunk-padded DRAM buffer or None.
            ag_out: destination table for the pipelined AllGather (3 chunks),
            sourced from contrib (or the JK jcontrib for fused convs).
            jkargs = (Wlin, jbcol, hout, jcontrib, pool_sb): fused JK+pool."""
            if not hasattr(conv, "qctr"):
                conv.qctr = 0
            pool_ps = None
            pool_sb = None
            if jkargs is not None:
                pool_ps = ps_pool.tile([P, G], F32, name="pool_ps")
                pool_sb = jkargs[4]
                jkargs = jkargs[:4]
            ag_src = contrib if contrib is not None else (
                jkargs[3] if jkargs is not None else None)
            if isinstance(tab, list):
                views = [t[:] for t in tab]
            else:
                views = [tab[k * CHUNK_T:(k + 1) * CHUNK_T] for k in range(3)]
            ag_fired = 0
            for g in ginfo:
                agg4 = ps_agg.tile([P, GW, P], F32, name="agg4")
                wslot = {wv: i for i, wv in enumerate(g["grp"])}
                for k in range(NSEC):
                    sk = g["secs"][k]
                    n = sk["n"]
                    g_t = sb_g.tile([P, SECMAX, P], DT, name="g_t", tag="g_t")
                    nch = (n + 9215) // 9216
                    step = ((n // nch) // P) * P
                    off = 0
                    while off < n:
                        ln = step if off + step + step <= n else n - off
                        nc.gpsimd.dma_gather(
                            g_t[:, off // P:(off + ln) // P, :],
                            views[k], idx_t[:, sk["icol"] + off // 16:sk["icol"] + (off + ln) // 16],
                            ln, ln, P, single_packet=False,
                            queue_num=conv.qctr % 4)
                        conv.qctr += 1
                        off += ln
                    i2g = sb_i2.tile([P, I2MAX, P], DT, name="i2g", tag="i2g")
                    if sk["nmm"]:
                        nc.sync.dma_start(i2g[:, 0:sk["nmm"], :],
                                          ind2p[:, sk["mi0"]:sk["mi0"] + sk["nmm"], :])
                    for t in range(sk["ntiles"]):
                        bs_ps = ps_bs.tile([P, P], F32, name="bs_ps")
                        for jj in range(4):
                            nc.tensor.matmul(
                                bs_ps[32 * jj:32 * jj + 32, :], o4_t[:],
                                g_t[:, t * 4 + jj, :],
                                start=True, stop=True, tile_position=(0, 32 * jj))
                        bs_sb = sb_bs.tile([P, P], DT, name="bs_sb")
                        nc.scalar.copy(bs_sb[:], bs_ps[:])
                        for (tt, wv, b0, b1, mi) in sk["mms"]:
                            if tt != t:
                                continue
                            agg = agg4[:, wslot[wv], :]
                            nc.tensor.matmul(agg, bs_sb[:], i2g[:, mi - sk["mi0"], :],
                                             start=(mi == g["first"][wv]),
                                             stop=(mi == g["last"][wv]))
                            if mi == g["last"][wv]:
                                win_tail(wv, agg, in_feat, out_feat, Wl, Wr,
                                         bcol, contrib, jkargs, pool_ps)
                                # pipelined AllGather chunks (after the
                                # threshold window's contrib write exists)
                                if ag_out is not None and ag_src is not None:
                                    while (ag_fired < 3
                                           and wv + 1 >= AGW[ag_fired]):
                                        kk = ag_fired
                                        nc.gpsimd.collective_compute(
                                            "AllGather", Alu.bypass,
                                            ins=[ag_src[kk * CHKB:(kk + 1) * CHKB, :]],
                                            outs=[ag_out[kk][:]],
                                            replica_groups=[list(range(NCORES))])
                                        ag_fired += 1
            if jkargs is not None:
                nc.vector.tensor_copy(pool_sb[:], pool_ps[:])

        # ---------------- block 0
        conv(xt, featA, featB, wt["b0_Wl1"], wt["b0_Wr1"], wt["b0_b1"],
             cA, ag_out=tabA)   # h1
        if n_convs >= 2:
            p0_sb = sb.tile([P, G], F32, name="p0_sb", tag="p0_sb")
            conv(tabA, featB, featC, wt["b0_Wl2"], wt["b0_Wr2"], wt["b0_b2"], None,
                 ag_out=tabB,
                 jkargs=(wt["b0_Wlin"], wt["b0_blin"], featA, cB, p0_sb))  # h2 + jk
        if n_convs >= 3:
            conv(tabB, featA, featB, wt["b1_Wl1"], wt["b1_Wr1"], wt["b1_b1"],
                 cA, ag_out=tabC)  # h1'
        if n_convs >= 4:
            p1_sb = sb.tile([P, G], F32, name="p1_sb", tag="p1_sb")
            conv(tabC, featB, featC, wt["b1_Wl2"], wt["b1_Wr2"], wt["b1_b2"], None,
                 jkargs=(wt["b1_Wlin"], wt["b1_blin"], featA, None, p1_sb))

            # ---------------- pooling allreduce + head
            nc.sync.dma_start(pc_in[:, 0:G], p0_sb[:])
            nc.sync.dma_start(pc_in[:, G:2 * G], p1_sb[:])
            nc.gpsimd.collective_compute(
                "AllReduce", Alu.add, ins=[pc_in[:]], outs=[pc_out[:]],
                replica_groups=[list(range(NCORES))])
            pools_sb = sb.tile([P, 2 * G], F32, name="pools_sb", tag="pools_sb")
            nc.sync.dma_start(pools_sb[:], pc_out[:])

            gbn = sb.tile([P, 2, G], F32, name="gbn", tag="gbn")
            for k in range(2):
                nc.vector.tensor_scalar(gbn[:, k, :], pools_sb[:, k * G:(k + 1) * G],
                                        bns_t[:, k:k + 1], bnt_t[:, k:k + 1],
                                        Alu.mult, Alu.add)
            l1_ps = ps_mm.tile([P, G], F32, name="l1_ps", tag="mm")
            for k in range(2):
                nc.tensor.matmul(l1_ps[:], l1w_t[:, k, :], gbn[:, k, :],
                                 start=(k == 0), stop=(k == 1))
            z1 = sb.tile([P, G], F32, name="z1", tag="z1")
            nc.vector.tensor_scalar(z1[:], l1_ps[:], l1b_t[:], 0.0, Alu.add, Alu.max)
            l2_ps = ps_mm.tile([P, G], F32, name="l2_ps", tag="mm")
            nc.tensor.matmul(l2_ps[0:C, :], l2w_t[:], z1[:], start=True, stop=True)
            z2 = sb.tile([P, G], F32, name="z2", tag="z2")
            nc.vector.tensor_scalar(z2[0:C, :], l2_ps[0:C, :], l2b_t[0:C, :], None, Alu.add)

            for half in range(2):
                zt_ps = ps_mm.tile([P, C], F32, name="zt_ps", tag="mm")
                nc.tensor.transpose(zt_ps[:, 0:C], z2[0:C, half * P:(half + 1) * P], id32_t[0:C, 0:C])
                znm = sb.tile([P, C], F32, name=f"znm{half}", tag=f"znm{half}")
                nc.vector.tensor_copy(znm[:], zt_ps[:, 0:C])
                nmax = sb.tile([P, 1], F32, name=f"nmax{half}", tag=f"nmax{half}")
                nc.vector.tensor_reduce(nmax[:], znm[:], mybir.AxisListType.X, Alu.max, negate=True)
                e_t = sb.tile([P, C], F32, name=f"e_t{half}", tag=f"e_t{half}")
                nc.scalar.activation(e_t[:], znm[:], Act.Exp,
                                     bias=nmax[:], scale=1.0)
                ssum = sb.tile([P, 1], F32, name=f"ssum{half}", tag=f"ssum{half}")
                nc.vector.tensor_reduce(ssum[:], e_t[:], mybir.AxisListType.X, Alu.add)
                rcp = sb.tile([P, 1], F32, name=f"rcp{half}", tag=f"rcp{half}")
                nc.vector.reciprocal(rcp[:], ssum[:])
                sm = sb.tile([P, C], F32, name=f"sm{half}", tag=f"sm{half}")
                nc.vector.tensor_scalar(sm[:], e_t[:], rcp[:], None, Alu.mult)
                nc.sync.dma_start(out[half * P:(half + 1) * P, :], sm[:])

        if debug_tables:
            for k in range(3):
                nc.sync.dma_start(dbgA[k * CHUNK_T:(k + 1) * CHUNK_T, :], tabA[k][:])
                if n_convs >= 2:
                    nc.sync.dma_start(dbgB[k * CHUNK_T:(k + 1) * CHUNK_T, :], tabB[k][:])

    nc.compile()
    return nc


# ------------------------------------------------------------------ runtime
def _install_profile_hook():
    try:
        from trn_agent_boot.trn_boot import _ntff_profile_via_ctypes
        hook = _ntff_profile_via_ctypes("/opt/axon/libaxon_pjrt.so")
        m = types.ModuleType("antenv.axon_hooks")
        m.get_axon_ntff_profile_hook = lambda: hook
        sys.modules.setdefault("antenv.axon_hooks", m)
    except Exception:
        pass


def kernel(**inputs):
    from concourse.bass_utils import run_bass_kernel_spmd

    n_convs = int(os.environ.get("KNC_CONVS", "4"))
    debug_tables = bool(int(os.environ.get("KDBG", "0")))
    trace = bool(int(os.environ.get("KTRACE", "0")))
    if trace:
        _install_profile_hook()

    sched, in_maps = _host_inputs(inputs)

    key = (n_convs, debug_tables, sched["key"])
    nc = _prog_cache.get(key)
    if nc is None:
        nc = _build_program(sched, n_convs=n_convs, debug_tables=debug_tables)
        _prog_cache[key] = nc

    res = run_bass_kernel_spmd(nc, in_maps, list(range(NCORES)), trace=trace)
    kernel.last_result = res
    out = res.results[0]["out"].astype(np.float32)
    return out


# revision 14
# speedup vs baseline: 1.1139x; 1.1139x over previous
"""GraphSAGE 2-block GNN (nn_BaselineModel_80607946211554) on 8 TRN2 NeuronCores.

Strategy: destination-node sharding, bf16 compute. Q7 SWDGE descriptor
generation is the hard bottleneck (~2.4ns/index + ~2.5us/instruction,
serialized on the GpSimd engine), so gathers are issued as few large
instructions: windows are batched into groups of 4 and each group's
lo/hi table section is fetched with a single dma_gather. Node tables
live in DRAM in a chunk-major layout (3 row-chunks of 2176 rows/core)
so the inter-conv AllGather can be fired in 3 pipelined pieces as soon
as the producing windows retire. Aggregation: ones-block stage-1 matmul
(sums per-dst slot quadruples, PE column-tiled) + streamed indicator
stage-2 matmul; PSUM->SBUF casts and bias+relu run on the Scalar (ACT)
engine to keep DVE free. SAGE linears run feature-major (weights
stationary); JK linear + graph pooling are fused into conv tails. The
tiny MLP head + softmax runs redundantly on every core in fp32.

Self-contained: hardcodes all shapes for the fixed problem instance.
"""
import os
import sys
import types
import numpy as np

N = 50000
E = 1600000
G = 256
F = 128
HID = 128
C = 10
NCORES = 8
NPC = N // NCORES            # 6250 nodes per core
P = 128
NW = (NPC + P - 1) // P      # 49 dst windows per core
GW = 4                       # windows per gather group
EPS = 1e-5

# chunk-major table layout: 3 chunks per core, CHKB rows each
CHKB = 2176                  # buffer rows per (core, chunk)
CHKR = (2048, 2048, NPC - 4096)   # real rows per chunk (2048, 2048, 2154)
CSLAB = 3 * CHKB             # 6528 contrib rows per core
NT = NCORES * CHKB * 3       # 52224 table rows
CHUNK_T = NCORES * CHKB      # 17408 table rows per chunk
ZROW = (2048, 2048, CHKR[2])   # guaranteed-zero chunk-local row per chunk
AGW = (16, 32, NW)           # window counts gating the 3 AllGather chunks
NSEC = 3                     # gather sections per window (one per table chunk)

_prog_cache = {}


def _bf16(a):
    import ml_dtypes
    return np.asarray(a, np.float32).astype(ml_dtypes.bfloat16)


def _trow(s):
    """Global node id -> chunk-major table row (vectorized)."""
    c, r = s // NPC, s % NPC
    k = np.minimum(r >> 11, 2)
    rr = r - (k << 11)
    return k * CHUNK_T + c * CHKB + rr


def _wrap_idx(sec):
    """int16 row values (len mult of 16) -> [128, n/16] wrapped layout."""
    n = len(sec)
    arr = sec.reshape(n // 16, 16).T.astype(np.int16)
    return np.tile(arr, (8, 1))


# ----------------------------------------------------------------- host prep
def _build_schedule(src, dst, invd_full):
    """Static schedule shared by all 4 convs + per-core gather/indicator data.

    Group layout (GW windows): [L(w0)..L(w3) | H(w0)..H(w3)], each window
    section per-dst padded to x4 then to x128 (pads gather a zero row).
    Stage-1 sums slot quadruples into blocks; stage-2 is one indicator
    matmul per (128-block tile x window segment)."""
    groups = [list(range(w0, min(w0 + GW, NW))) for w0 in range(0, NW, GW)]

    # --- per-core edge prep
    core_data = []
    nS = np.zeros((NCORES, NW, NSEC), np.int64)
    for c in range(NCORES):
        m = (dst >= c * NPC) & (dst < (c + 1) * NPC)
        s = src[m].astype(np.int64)
        d = (dst[m] - c * NPC).astype(np.int64)
        t = _trow(s)
        sec = t // CHUNK_T
        val = t % CHUNK_T
        w = d >> 7
        order = np.lexsort((d, sec, w))
        s, d, sec, w, val = s[order], d[order], sec[order], w[order], val[order]
        cnt = np.bincount(d * NSEC + sec, minlength=NPC * NSEC).reshape(NPC, NSEC)
        pl = ((cnt + 3) >> 2) << 2          # per-(dst,section) slots, x4
        plp = np.zeros((NW * P, NSEC), np.int64)
        plp[:NPC] = pl
        plw = plp.reshape(NW, P, NSEC).sum(1)
        nS[c] = ((plw + 127) >> 7) << 7
        core_data.append((s, d, sec, w, val, pl))

    nSm = np.maximum(nS.max(0), P)   # [NW, NSEC] shared across cores

    # --- group geometry (shared); sections padded to x512 slots
    ginfo = []
    icol = 0
    mm_total = 0
    for grp in groups:
        secs = []
        for k in range(NSEC):
            raw = int(nSm[grp, k].sum())
            n = ((raw + 511) // 512) * 512
            ncols = n // P
            ntiles = ncols // 4
            colw = []
            for w in grp:
                colw += [w] * (int(nSm[w, k]) // P)
            colw += [-1] * (ncols - len(colw))
            blkw = np.repeat(np.array(colw, np.int64), 32)
            mms = []
            for t in range(ntiles):
                bw = blkw[t * 128:(t + 1) * 128]
                runs = []
                prev = None
                for b in range(128):
                    if bw[b] != prev:
                        runs.append([int(bw[b]), b, b + 1])
                        prev = bw[b]
                    else:
                        runs[-1][2] = b + 1
                for wv, b0, b1 in runs:
                    if wv < 0:
                        continue
                    mms.append((t, wv, b0, b1, mm_total))
                    mm_total += 1
            secs.append(dict(n=n, raw=raw, ncols=ncols, ntiles=ntiles,
                             mms=mms, icol=icol,
                             mi0=mms[0][4] if mms else mm_total, nmm=len(mms)))
            icol += n // 16
        first = {}
        last = {}
        for sec in secs:
            for (t, wv, b0, b1, mi) in sec["mms"]:
                if wv not in first:
                    first[wv] = mi
                last[wv] = mi
        ginfo.append(dict(grp=grp, secs=secs, first=first, last=last))
    idx_cols = icol
    sec_cols = max(sec["ncols"] for g in ginfo for sec in g["secs"])
    i2max = max(sec["nmm"] for g in ginfo for sec in g["secs"])

    # --- per-core idx + indicator data
    per_core = []
    for c in range(NCORES):
        s, d, sec, w, val, pl = core_data[c]
        plw3 = np.zeros((NW * P, NSEC), np.int64)
        plw3[:NPC] = pl
        plw3 = plw3.reshape(NW, P, NSEC)
        dstart = np.cumsum(plw3, axis=1) - plw3      # [NW,128,NSEC]
        key = d * NSEC + sec
        if len(key):
            grp_change = np.r_[True, key[1:] != key[:-1]]
            first_pos = np.flatnonzero(grp_change)
            gidx = np.cumsum(grp_change) - 1
            rank = np.arange(len(d)) - first_pos[gidx]
        else:
            rank = np.zeros(0, np.int64)
        pos = dstart[w, d & 127, sec] + rank         # slot within section

        idx_arr = np.zeros((P, idx_cols), np.int16)
        ind2 = np.zeros((P, mm_total, P), np.float32)   # [block, mm, dst]
        for gi, g in enumerate(ginfo):
            grp = g["grp"]
            for k in range(NSEC):
                sk = g["secs"][k]
                off = {}
                o = 0
                for wv in grp:
                    off[wv] = o
                    o += int(nSm[wv, k])
                seck = np.full(sk["n"], ZROW[k], np.int64)
                for wv in grp:
                    mk = (w == wv) & (sec == k)
                    seck[off[wv] + pos[mk]] = val[mk]
                idx_arr[:, sk["icol"]:sk["icol"] + sk["n"] // 16] = _wrap_idx(seck)

                b2d = np.full(sk["ntiles"] * 128, -1, np.int64)
                bo = 0
                for wv in grp:
                    d0, d1 = wv * P, min((wv + 1) * P, NPC)
                    dloc = np.arange(d1 - d0)
                    bK = np.repeat(dloc, pl[d0:d1, k] // 4)
                    b2d[bo:bo + len(bK)] = bK
                    bo += int(nSm[wv, k]) // 4
                for (t, wv, b0, b1, mi) in sk["mms"]:
                    bs = b2d[t * 128 + b0:t * 128 + b1]
                    rows = np.arange(b0, b1)
                    vm = bs >= 0
                    ind2[rows[vm], mi, bs[vm]] = invd_full[
                        c * NPC + wv * P + bs[vm]]
        per_core.append(dict(idx=idx_arr, ind2=_bf16(ind2)))

    sched = dict(ginfo=ginfo, idx_cols=idx_cols, mm_total=mm_total,
                 sec_cols=sec_cols, i2max=i2max,
                 key=(idx_cols, mm_total, sec_cols, i2max,
                      tuple(int(x) for x in nSm.reshape(-1))))
    return sched, per_core


def _host_inputs(inputs):
    f32 = lambda a: np.asarray(a, np.float32)
    x = f32(inputs["x"])
    ei = np.asarray(inputs["edge_index"], np.int64)
    batch = np.asarray(inputs["batch"], np.int64)
    src, dst = ei[0], ei[1]

    deg = np.bincount(dst, minlength=N).astype(np.float32)
    invd_full = (1.0 / np.maximum(deg, 1.0)).astype(np.float32)

    sched, per_core = _build_schedule(src, dst, invd_full)

    # x in chunk-major table layout
    xt = np.zeros((NT, F), np.float32)
    xt[_trow(np.arange(N))] = x

    o4 = np.zeros((P, 32), np.float32)
    for e in range(P):
        o4[e, e // 4] = 1.0
    ident = np.eye(P, dtype=np.float32)

    # BN folding
    s_bn = f32(inputs["bn_gamma"]) / np.sqrt(f32(inputs["bn_rv"]) + EPS)
    t_bn = f32(inputs["bn_beta"]) - f32(inputs["bn_rm"]) * s_bn
    bns2 = s_bn.reshape(2, P).T.copy()     # [128, 2]
    bnt2 = t_bn.reshape(2, P).T.copy()

    shared = {
        "xt": _bf16(xt), "o4": _bf16(o4), "ident": _bf16(ident),
        "ident32": ident,
        "bns2": bns2, "bnt2": bnt2,
        "l1w": f32(inputs["lin1_W"]), "l1b": f32(inputs["lin1_b"]),
        "l2w": f32(inputs["lin2_W"]), "l2b": f32(inputs["lin2_b"]),
    }
    for b in (0, 1):
        for nm in ("Wl1", "Wr1", "Wl2", "Wr2", "Wlin"):
            shared[f"b{b}_{nm}"] = _bf16(inputs[f"b{b}_{nm}"])
        for nm in ("b1", "b2", "blin"):
            shared[f"b{b}_{nm}"] = f32(inputs[f"b{b}_{nm}"])

    in_maps = []
    for c in range(NCORES):
        xoT = np.zeros((F, NW * P), np.float32)
        xoT[:, :NPC] = x[c * NPC:(c + 1) * NPC].T
        pool_ind = np.zeros((NW, P, G), np.float32)
        bt = batch[c * NPC:(c + 1) * NPC]
        btp = np.full(NW * P, -1, np.int64)
        btp[:NPC] = bt
        btp2 = btp.reshape(NW, P)
        for wi in range(NW):
            vmb = btp2[wi] >= 0
            pool_ind[wi, np.arange(P)[vmb], btp2[wi][vmb]] = 1.0
        im = dict(shared)
        im.update({
            "xoT": _bf16(xoT), "poolind": _bf16(pool_ind),
            "idx": per_core[c]["idx"], "ind2": per_core[c]["ind2"],
        })
        in_maps.append(im)
    return sched, in_maps


# ------------------------------------------------------------- bass program
def _build_program(sched, n_convs=4, debug_tables=False):
    import concourse.bass as bass
    import concourse.mybir as mybir
    import concourse.tile as tile
    from concourse import bacc
    from concourse import library_config
    from contextlib import ExitStack

    dt = mybir.dt
    DT = dt.bfloat16
    F32 = dt.float32
    Alu = mybir.AluOpType
    Act = mybir.ActivationFunctionType

    ginfo = sched["ginfo"]
    SECMAX = sched["sec_cols"]
    I2MAX = sched["i2max"]

    nc = bacc.Bacc("TRN2", debug=False, num_swdge_queues=4)

    # ---- parameters
    xt = nc.declare_dram_parameter("xt", [NT, F], DT, isOutput=False)
    xoT = nc.declare_dram_parameter("xoT", [F, NW * P], DT, isOutput=False)
    idxp = nc.declare_dram_parameter("idx", [P, sched["idx_cols"]], dt.int16, isOutput=False)
    ind2p = nc.declare_dram_parameter("ind2", [P, sched["mm_total"], P], DT, isOutput=False)
    poolp = nc.declare_dram_parameter("poolind", [NW, P, G], DT, isOutput=False)
    o4p = nc.declare_dram_parameter("o4", [P, 32], DT, isOutput=False)
    identp = nc.declare_dram_parameter("ident", [P, P], DT, isOutput=False)
    ident32p = nc.declare_dram_parameter("ident32", [P, P], F32, isOutput=False)
    wp = {}
    for b in (0, 1):
        for nm, shp, dty in (("Wl1", [F, HID], DT), ("Wr1", [F, HID], DT),
                             ("b1", [HID], F32),
                             ("Wl2", [HID, HID], DT), ("Wr2", [HID, HID], DT),
                             ("b2", [HID], F32),
                             ("Wlin", [2 * HID, HID], DT), ("blin", [HID], F32)):
            wp[f"b{b}_{nm}"] = nc.declare_dram_parameter(f"b{b}_{nm}", shp, dty, isOutput=False)
    bns2p = nc.declare_dram_parameter("bns2", [P, 2], F32, isOutput=False)
    bnt2p = nc.declare_dram_parameter("bnt2", [P, 2], F32, isOutput=False)
    l1wp = nc.declare_dram_parameter("l1w", [2 * HID, HID], F32, isOutput=False)
    l1bp = nc.declare_dram_parameter("l1b", [HID], F32, isOutput=False)
    l2wp = nc.declare_dram_parameter("l2w", [HID, C], F32, isOutput=False)
    l2bp = nc.declare_dram_parameter("l2b", [C], F32, isOutput=False)

    out = nc.declare_dram_parameter("out", [G, C], F32, isOutput=True)
    if debug_tables:
        dbgA = nc.declare_dram_parameter("dbgA", [NT, F], DT, isOutput=True)
        dbgB = nc.declare_dram_parameter("dbgB", [NT, F], DT, isOutput=True)

    with tile.TileContext(nc) as tc, ExitStack() as ctx:
        sb = ctx.enter_context(tc.tile_pool(name="sb", bufs=1))
        sb_feat = ctx.enter_context(tc.tile_pool(name="sb_feat", bufs=1))
        sb_g = ctx.enter_context(tc.tile_pool(name="sb_g", bufs=6))
        sb_i2 = ctx.enter_context(tc.tile_pool(name="sb_i2", bufs=4))
        sb_bs = ctx.enter_context(tc.tile_pool(name="sb_bs", bufs=6))
        sb_ms = ctx.enter_context(tc.tile_pool(name="sb_ms", bufs=3))
        ps_bs = ctx.enter_context(tc.tile_pool(name="ps_bs", bufs=3, space="PSUM"))
        ps_agg = ctx.enter_context(tc.tile_pool(name="ps_agg", bufs=2, space="PSUM"))
        ps_mm = ctx.enter_context(tc.tile_pool(name="ps_mm", bufs=2, space="PSUM"))
        ps_pool = ctx.enter_context(tc.tile_pool(name="ps_pool", bufs=1, space="PSUM"))
        dram = ctx.enter_context(tc.tile_pool(name="dram", bufs=1, space="DRAM"))

        nc.gpsimd.load_library(library_config.mlp)

        # ---- constants into SBUF
        o4_t = sb.tile([P, 32], DT, name="o4_t", tag="o4_t")
        nc.sync.dma_start(o4_t[:], o4p[:])
        id_t = sb.tile([P, P], DT, name="id_t", tag="id_t")
        nc.sync.dma_start(id_t[:], identp[:])
        id32_t = sb.tile([P, P], F32, name="id32_t", tag="id32_t")
        nc.sync.dma_start(id32_t[:], ident32p[:])
        idx_t = sb.tile([P, sched["idx_cols"]], dt.int16, name="idx_t", tag="idx_t")
        nc.sync.dma_start(idx_t[:], idxp[:])


        wt = {}
        for b in (0, 1):
            for nm in ("Wl1", "Wr1", "Wl2", "Wr2"):
                w_t = sb.tile([P, P], DT, name=f"w{b}{nm}", tag=f"w{b}{nm}")
                nc.sync.dma_start(w_t[:], wp[f"b{b}_{nm}"][:])
                wt[f"b{b}_{nm}"] = w_t
            wlin_t = sb.tile([P, 2, P], DT, name=f"w{b}lin", tag=f"w{b}lin")
            nc.sync.dma_start(wlin_t[:, 0, :], wp[f"b{b}_Wlin"][0:P, :])
            nc.sync.dma_start(wlin_t[:, 1, :], wp[f"b{b}_Wlin"][P:2 * P, :])
            wt[f"b{b}_Wlin"] = wlin_t
            for nm in ("b1", "b2", "blin"):
                b_t = sb.tile([P, 1], F32, name=f"b{b}{nm}", tag=f"b{b}{nm}")
                nc.sync.dma_start(b_t[:], wp[f"b{b}_{nm}"][:, None])
                wt[f"b{b}_{nm}"] = b_t
        bns_t = sb.tile([P, 2], F32, name="bns_t", tag="bns_t")
        nc.sync.dma_start(bns_t[:], bns2p[:])
        bnt_t = sb.tile([P, 2], F32, name="bnt_t", tag="bnt_t")
        nc.sync.dma_start(bnt_t[:], bnt2p[:])
        l1w_t = sb.tile([P, 2, P], F32, name="l1w_t", tag="l1w_t")
        nc.sync.dma_start(l1w_t[:, 0, :], l1wp[0:P, :])
        nc.sync.dma_start(l1w_t[:, 1, :], l1wp[P:2 * P, :])
        l1b_t = sb.tile([P, 1], F32, name="l1b_t", tag="l1b_t")
        nc.sync.dma_start(l1b_t[:], l1bp[:, None])
        l2w_t = sb.tile([P, C], F32, name="l2w_t", tag="l2w_t")
        nc.sync.dma_start(l2w_t[:], l2wp[:])
        l2b_t = sb.tile([P, 1], F32, name="l2b_t", tag="l2b_t")
        nc.sync.dma_start(l2b_t[0:C, :], l2bp[:, None])

        # feature-major activation buffers [128, NW*128] bf16
        featA = sb_feat.tile([P, NW * P], DT, name="featA", tag="featA")
        featB = sb_feat.tile([P, NW * P], DT, name="featB", tag="featB")
        featC = sb_feat.tile([P, NW * P], DT, name="featC", tag="featC")
        nc.sync.dma_start(featA[:], xoT[:])

        zero_t = sb.tile([P, P], DT, name="zero_t", tag="zero_t")
        nc.vector.memset(zero_t[:], 0.0)

        # DRAM scratch: contribs (chunk-padded) + tables
        cA = dram.tile([CSLAB, F], DT, name="cA", tag="cA")
        cB = dram.tile([CSLAB, F], DT, name="cB", tag="cB")
        tabA = [dram.tile([CHUNK_T, F], DT, addr_space="Shared",
                          name=f"tabA{k}", tag=f"tabA{k}") for k in range(3)]
        tabB = [dram.tile([CHUNK_T, F], DT, addr_space="Shared",
                          name=f"tabB{k}", tag=f"tabB{k}") for k in range(3)]
        tabC = [dram.tile([CHUNK_T, F], DT, addr_space="Shared",
                          name=f"tabC{k}", tag=f"tabC{k}") for k in range(3)]
        pc_in = dram.tile([P, 2 * G], F32, name="pc_in", tag="pc_in")
        pc_out = dram.tile([P, 2 * G], F32, addr_space="Shared", name="pc_out", tag="pc_out")
        # zero the per-chunk pad rows of the contribs
        for cbuf in (cA, cB):
            for k in range(3):
                z0, z1 = k * CHKB + CHKR[k], (k + 1) * CHKB
                r = z0
                while r < z1:
                    n = min(P, z1 - r)
                    nc.sync.dma_start(cbuf[r:r + n, :], zero_t[0:n, :])
                    r += n

        # tiny warmup collective
        wu_in = dram.tile([P, 8], DT, name="wu_in", tag="wu_in")
        wu_out = dram.tile([P, 64], DT, addr_space="Shared", name="wu_out", tag="wu_out")
        nc.sync.dma_start(wu_in[0:P, :], zero_t[0:P, 0:8])
        nc.gpsimd.collective_compute(
            "AllGather", Alu.bypass, ins=[wu_in[:]], outs=[wu_out[:]],
            replica_groups=[list(range(NCORES))])

        def win_tail(w, agg, in_feat, out_feat, Wl, Wr, bcol, contrib, jkargs,
                     pool_ps):
            """Everything after window w's aggregation closes."""
            mT_sb = sb_ms.tile([P, P], DT, name="mT_sb", tag="mT_sb")
            nc.scalar.copy(mT_sb[:], agg[:])
            h_ps = ps_mm.tile([P, P], F32, name="h_ps", tag="mm")
            nc.tensor.matmul(h_ps[:], Wl[:], mT_sb[:], start=True, stop=False)
            nc.tensor.matmul(h_ps[:], Wr[:], in_feat[:, w * P:(w + 1) * P], start=False, stop=True)
            nc.scalar.activation(out_feat[:, w * P:(w + 1) * P], h_ps[:], Act.Relu, bias=bcol[:], scale=1.0)
            if contrib is not None:
                rows = min(P, NPC - w * P)
                k = 0 if w < 16 else (1 if w < 32 else 2)
                crow = k * CHKB + (w * P - k * 2048)
                hnm_ps = ps_mm.tile([P, P], DT, name="hnm_ps", tag="mm")
                nc.tensor.transpose(hnm_ps[:], out_feat[:, w * P:(w + 1) * P], id_t[:])
                hnm_sb = sb_ms.tile([P, P], DT, name="hnm_sb", tag="hnm_sb")
                nc.vector.tensor_copy(hnm_sb[:], hnm_ps[:])
                nc.scalar.dma_start(contrib[crow:crow + rows, :], hnm_sb[0:rows, :])
            if jkargs is not None:
                Wlin, jbcol, hout, jcontrib = jkargs
                j_ps = ps_mm.tile([P, P], F32, name="jk_ps", tag="mm")
                nc.tensor.matmul(j_ps[:], Wlin[:, 0, :], in_feat[:, w * P:(w + 1) * P], start=True, stop=False)
                nc.tensor.matmul(j_ps[:], Wlin[:, 1, :], out_feat[:, w * P:(w + 1) * P], start=False, stop=True)
                nc.scalar.activation(hout[:, w * P:(w + 1) * P], j_ps[:], Act.Relu, bias=jbcol[:], scale=1.0)
                jnm_ps = ps_mm.tile([P, P], DT, name="jknm_ps", tag="mm")
                nc.tensor.transpose(jnm_ps[:], hout[:, w * P:(w + 1) * P], id_t[:])
                jnm_sb = sb_ms.tile([P, P], DT, name="jknm_sb", tag="jknm_sb")
                nc.vector.tensor_copy(jnm_sb[:], jnm_ps[:])
                if jcontrib is not None:
                    rows = min(P, NPC - w * P)
                    k = 0 if w < 16 else (1 if w < 32 else 2)
                    crow = k * CHKB + (w * P - k * 2048)
                    nc.scalar.dma_start(jcontrib[crow:crow + rows, :], jnm_sb[0:rows, :])
                pind = sb_i2.tile([P, G], DT, name="pind", tag="pind")
                nc.sync.dma_start(pind[:], poolp[w])
                nc.tensor.matmul(pool_ps[:], jnm_sb[:], pind[:],
                                 start=(w == 0), stop=(w == NW - 1))

        def conv(tab, in_feat, out_feat, Wl, Wr, bcol, contrib, ag_out=None,
                 jkargs=None, pending_ag=None):
            """One SAGE conv. contrib: chunk-padded DRAM buffer or None.
            ag_out: per-chunk AllGather destinations; chunks 0/1 fire after
            groups 7/11 (delayed so the contrib deps are met when the
            instruction reaches the gpsimd queue head); chunk 2 is returned
            as a closure that the NEXT conv fires after its first section-0/1
            gathers (cross-conv stitch). pending_ag: the previous conv's
            chunk-2 closure."""
            if not hasattr(conv, "qctr"):
                conv.qctr = 0
            pool_ps = None
            pool_sb = None
            if jkargs is not None:
                pool_ps = ps_pool.tile([P, G], F32, name="pool_ps")
                pool_sb = jkargs[4]
                jkargs = jkargs[:4]
            ag_src = contrib if contrib is not None else (
                jkargs[3] if jkargs is not None else None)
            if isinstance(tab, list):
                views = [t[:] for t in tab]
            else:
                views = [tab[k * CHUNK_T:(k + 1) * CHUNK_T] for k in range(3)]

            def ag_fire(kk):
                nc.gpsimd.collective_compute(
                    "AllGather", Alu.bypass,
                    ins=[ag_src[kk * CHKB:(kk + 1) * CHKB, :]],
                    outs=[ag_out[kk][:]],
                    replica_groups=[list(range(NCORES))])

            gtiles = {}

            def emit_gather(gi, k):
                sk = ginfo[gi]["secs"][k]
                n = sk["n"]
                g_t = sb_g.tile([P, SECMAX, P], DT, name="g_t", tag="g_t")
                gtiles[(gi, k)] = g_t
                nch = (n + 9215) // 9216
                step = ((n // nch) // P) * P
                off = 0
                while off < n:
                    ln = step if off + step + step <= n else n - off
                    nc.gpsimd.dma_gather(
                        g_t[:, off // P:(off + ln) // P, :],
                        views[k], idx_t[:, sk["icol"] + off // 16:sk["icol"] + (off + ln) // 16],
                        ln, ln, P, single_packet=False,
                        queue_num=conv.qctr % 4)
                    conv.qctr += 1
                    off += ln

            pre = [(0, 0), (0, 1), (1, 0), (1, 1)] if pending_ag is not None else []
            for (gi, k) in pre:
                emit_gather(gi, k)
            if pending_ag is not None:
                pending_ag()

            for gi, g in enumerate(ginfo):
                for k in range(NSEC):
                    if (gi, k) not in pre:
                        emit_gather(gi, k)
                if ag_out is not None and ag_src is not None:
                    if gi == 7:
                        ag_fire(0)
                    elif gi == 11:
                        ag_fire(1)
                agg4 = ps_agg.tile([P, GW, P], F32, name="agg4")
                wslot = {wv: i for i, wv in enumerate(g["grp"])}
                for k in range(NSEC):
                    sk = g["secs"][k]
                    g_t = gtiles.pop((gi, k))
                    i2g = sb_i2.tile([P, I2MAX, P], DT, name="i2g", tag="i2g")
                    if sk["nmm"]:
                        nc.sync.dma_start(i2g[:, 0:sk["nmm"], :],
                                          ind2p[:, sk["mi0"]:sk["mi0"] + sk["nmm"], :])
                    for t in range(sk["ntiles"]):
                        bs_ps = ps_bs.tile([P, P], F32, name="bs_ps")
                        for jj in range(4):
                            nc.tensor.matmul(
                                bs_ps[32 * jj:32 * jj + 32, :], o4_t[:],
                                g_t[:, t * 4 + jj, :],
                                start=True, stop=True, tile_position=(0, 32 * jj))
                        bs_sb = sb_bs.tile([P, P], DT, name="bs_sb")
                        nc.scalar.copy(bs_sb[:], bs_ps[:])
                        for (tt, wv, b0, b1, mi) in sk["mms"]:
                            if tt != t:
                                continue
                            agg = agg4[:, wslot[wv], :]
                            nc.tensor.matmul(agg, bs_sb[:], i2g[:, mi - sk["mi0"], :],
                                             start=(mi == g["first"][wv]),
                                             stop=(mi == g["last"][wv]))
                            if mi == g["last"][wv]:
                                win_tail(wv, agg, in_feat, out_feat, Wl, Wr,
                                         bcol, contrib, jkargs, pool_ps)
            if jkargs is not None:
                nc.vector.tensor_copy(pool_sb[:], pool_ps[:])
            if ag_out is not None and ag_src is not None:
                return lambda: ag_fire(2)
            return None

        # ---------------- block 0
        ag2 = conv(xt, featA, featB, wt["b0_Wl1"], wt["b0_Wr1"], wt["b0_b1"],
                   cA, ag_out=tabA)   # h1
        if n_convs >= 2:
            p0_sb = sb.tile([P, G], F32, name="p0_sb", tag="p0_sb")
            ag2 = conv(tabA, featB, featC, wt["b0_Wl2"], wt["b0_Wr2"], wt["b0_b2"], None,
                       ag_out=tabB, pending_ag=ag2,
                       jkargs=(wt["b0_Wlin"], wt["b0_blin"], featA, cB, p0_sb))  # h2 + jk
        if n_convs >= 3:
            ag2 = conv(tabB, featA, featB, wt["b1_Wl1"], wt["b1_Wr1"], wt["b1_b1"],
                       cA, ag_out=tabC, pending_ag=ag2)  # h1'
        if n_convs >= 4:
            p1_sb = sb.tile([P, G], F32, name="p1_sb", tag="p1_sb")
            conv(tabC, featB, featC, wt["b1_Wl2"], wt["b1_Wr2"], wt["b1_b2"], None,
                 pending_ag=ag2,
                 jkargs=(wt["b1_Wlin"], wt["b1_blin"], featA, None, p1_sb))

            # ---------------- pooling allreduce + head
            nc.sync.dma_start(pc_in[:, 0:G], p0_sb[:])
            nc.sync.dma_start(pc_in[:, G:2 * G], p1_sb[:])
            nc.gpsimd.collective_compute(
                "AllReduce", Alu.add, ins=[pc_in[:]], outs=[pc_out[:]],
                replica_groups=[list(range(NCORES))])
            pools_sb = sb.tile([P, 2 * G], F32, name="pools_sb", tag="pools_sb")
            nc.sync.dma_start(pools_sb[:], pc_out[:])

            gbn = sb.tile([P, 2, G], F32, name="gbn", tag="gbn")
            for k in range(2):
                nc.vector.tensor_scalar(gbn[:, k, :], pools_sb[:, k * G:(k + 1) * G],
                                        bns_t[:, k:k + 1], bnt_t[:, k:k + 1],
                                        Alu.mult, Alu.add)
            l1_ps = ps_mm.tile([P, G], F32, name="l1_ps", tag="mm")
            for k in range(2):
                nc.tensor.matmul(l1_ps[:], l1w_t[:, k, :], gbn[:, k, :],
                                 start=(k == 0), stop=(k == 1))
            z1 = sb.tile([P, G], F32, name="z1", tag="z1")
            nc.vector.tensor_scalar(z1[:], l1_ps[:], l1b_t[:], 0.0, Alu.add, Alu.max)
            l2_ps = ps_mm.tile([P, G], F32, name="l2_ps", tag="mm")
            nc.tensor.matmul(l2_ps[0:C, :], l2w_t[:], z1[:], start=True, stop=True)
            z2 = sb.tile([P, G], F32, name="z2", tag="z2")
            nc.vector.tensor_scalar(z2[0:C, :], l2_ps[0:C, :], l2b_t[0:C, :], None, Alu.add)

            for half in range(2):
                zt_ps = ps_mm.tile([P, C], F32, name="zt_ps", tag="mm")
                nc.tensor.transpose(zt_ps[:, 0:C], z2[0:C, half * P:(half + 1) * P], id32_t[0:C, 0:C])
                znm = sb.tile([P, C], F32, name=f"znm{half}", tag=f"znm{half}")
                nc.vector.tensor_copy(znm[:], zt_ps[:, 0:C])
                nmax = sb.tile([P, 1], F32, name=f"nmax{half}", tag=f"nmax{half}")
                nc.vector.tensor_reduce(nmax[:], znm[:], mybir.AxisListType.X, Alu.max, negate=True)
                e_t = sb.tile([P, C], F32, name=f"e_t{half}", tag=f"e_t{half}")
                nc.scalar.activation(e_t[:], znm[:], Act.Exp,
                                     bias=nmax[:], scale=1.0)
                ssum = sb.tile([P, 1], F32, name=f"ssum{half}", tag=f"ssum{half}")
                nc.vector.tensor_reduce(ssum[:], e_t[:], mybir.AxisListType.X, Alu.add)
                rcp = sb.tile([P, 1], F32, name=f"rcp{half}", tag=f"rcp{half}")
                nc.vector.reciprocal(rcp[:], ssum[:])
                sm = sb.tile([P, C], F32, name=f"sm{half}", tag=f"sm{half}")
                nc.vector.tensor_scalar(sm[:], e_t[:], rcp[:], None, Alu.mult)
                nc.sync.dma_start(out[half * P:(half + 1) * P, :], sm[:])

        if debug_tables:
            for k in range(3):
                nc.sync.dma_start(dbgA[k * CHUNK_T:(k + 1) * CHUNK_T, :], tabA[k][:])
                if n_convs >= 2:
                    nc.sync.dma_start(dbgB[k * CHUNK_T:(k + 1) * CHUNK_T, :], tabB[k][:])

    nc.compile()
    return nc


# ------------------------------------------------------------------ runtime
def _install_profile_hook():
    try:
        from trn_agent_boot.trn_boot import _ntff_profile_via_ctypes
        hook = _ntff_profile_via_ctypes("/opt/axon/libaxon_pjrt.so")
        m = types.ModuleType("antenv.axon_hooks")
        m.get_axon_ntff_profile_hook = lambda: hook
        sys.modules.setdefault("antenv.axon_hooks", m)
    except Exception:
        pass


def kernel(**inputs):
    from concourse.bass_utils import run_bass_kernel_spmd

    n_convs = int(os.environ.get("KNC_CONVS", "4"))
    debug_tables = bool(int(os.environ.get("KDBG", "0")))
    trace = bool(int(os.environ.get("KTRACE", "0")))
    if trace:
        _install_profile_hook()

    sched, in_maps = _host_inputs(inputs)

    key = (n_convs, debug_tables, sched["key"])
    nc = _prog_cache.get(key)
    if nc is None:
        nc = _build_program(sched, n_convs=n_convs, debug_tables=debug_tables)
        _prog_cache[key] = nc

    res = run_bass_kernel_spmd(nc, in_maps, list(range(NCORES)), trace=trace)
    kernel.last_result = res
    out = res.results[0]["out"].astype(np.float32)
    return out
